# revision 32
# baseline (speedup 1.0000x reference)
"""Gated multi-head attention on 8 NeuronCores.

Sharding (hardcoded): core c -> (batch b = c // 4, head-group g = c % 4).
Data-parallel over B=2, tensor-parallel over the 16 heads in groups of 4.
Each core computes its 4 heads' attention plus the corresponding slice of
the output projection; the host sums the 4 head-group partials per batch
(bf16 on the wire, f32 accumulate) and adds the output bias.

Per-core kernel (all matmuls bf16):
  qT[256,2048] = (Wq_slice.T).T @ x_q.T   (gate sigmoid/sqrt(D) + bias folded
                                           into the PSUM->SBUF eviction, which
                                           runs on the vector engine so the
                                           scalar engine does exp only)
  kT[256,2048], v[2048,256] likewise (v in natural [pos, dim] layout, bf16,
                                      with a ones-column appended per head)
  per head h, per 512-wide query block:
    S^T[k,q] = kT_h.T-chunks @ qT_h      (K=64 contraction)
    P^T = exp(S^T)                       (no max-subtraction: logits ~ +-4)
    acc[q,65] = P^T-chunks.T @ [V_h | 1] (bf16; col 64 = softmax denominator)
    A[q, h*64:...] = acc[:, :64] * recip(acc[:, 64])
  y[q,1024] = A @ Wo_slice.T  via PE-transpose of A then bf16 matmul

Schedule: V-projection chunks are interleaved with unit-0's S^T chunks so
exp starts ~15us earlier; AV groups of earlier units fill PE gaps during
later units' exp phases; O-proj groups interleave with the last head's AV
groups so y stores drain early.
"""

import math
from contextlib import ExitStack

import numpy as np

import concourse.bass as bass
import concourse.tile as tile
from concourse import mybir
from concourse.bass_utils import run_bass_kernel_spmd
from concourse.masks import make_identity

B = 2
N = 2048
E = 1024
H = 16
D = 64
NCORES = 8
GROUPS = NCORES // B      # head-groups per batch
HG = H // GROUPS          # heads per core
DH = HG * D               # 256 head-dims per core
P = 128

F32 = mybir.dt.float32
BF16 = mybir.dt.bfloat16
AF = mybir.ActivationFunctionType
MUL = mybir.AluOpType.mult
ADD = mybir.AluOpType.add

TRACE = False
LAST_RESULTS = None


def _split_drain_waits(nc):
    """The installed walrus build accepts only ONE sync-wait per instruction
    (one NEURON_ISA_TPB_EVENTS slot), but Tile emits several on drains,
    matmuls, etc.  Hoist all but the last wait onto dedicated single-wait
    NOPs ahead of the instruction on the same engine (the lowering newer
    walrus performs itself)."""
    n = 0
    for fn in nc.m.functions:
        for bb in fn.blocks:
            insts = bb.instructions
            idx = 0
            while idx < len(insts):
                inst = insts[idx]
                si = inst.sync_info
                if si is not None and len(si.on_wait) > 1:
                    waits = list(si.on_wait)
                    nops = []
                    for w in waits[:-1]:
                        n += 1
                        nop = mybir.InstNoOp(
                            name=f"waitsplit-{n}",
                            engine=inst.engine,
                            sync_info=mybir.SyncInfo(on_wait=[w], on_update=[]),
                            bass_nofuse=True,
                        )
                        nc.register_instruction(nop)
                        nops.append(nop)
                    inst.sync_info = mybir.SyncInfo(
                        on_wait=[waits[-1]], on_update=list(si.on_update))
                    insts[idx:idx] = nops
                    idx += len(nops)
                idx += 1
    return n


def _build():
    nc = bass.Bass()
    # Weights are pre-packed on the host, chunk-major: [128, KC*DH] where
    # column kc*DH+j holds W.T[kc*128+p, j] -- each SBUF partition row is
    # one contiguous 4KB DMA line instead of KC strided 512B lines.
    xqT = nc.dram_tensor("xqT", [E, N], BF16, kind="ExternalInput")
    xkT = nc.dram_tensor("xkT", [E, N], BF16, kind="ExternalInput")
    xvT = nc.dram_tensor("xvT", [E, N], BF16, kind="ExternalInput")
    wqP = nc.dram_tensor("wqP", [P, (E // P) * DH], BF16, kind="ExternalInput")
    wkP = nc.dram_tensor("wkP", [P, (E // P) * DH], BF16, kind="ExternalInput")
    wvP = nc.dram_tensor("wvP", [P, (E // P) * DH], BF16, kind="ExternalInput")
    woP = nc.dram_tensor("woP", [P, (DH // P) * E], BF16, kind="ExternalInput")
    # scale/bias table, [128, 3*MC]: cols [qscale | qbias | kbias] x MC
    sbt = nc.dram_tensor("sbt", [P, 3 * (DH // P)], F32, kind="ExternalInput")
    vbias = nc.dram_tensor("vbias", [DH], F32, kind="ExternalInput")
    y = nc.dram_tensor("y", [N, E], BF16, kind="ExternalOutput")

    KC = E // P            # 8 contraction chunks over the embed dim
    MC = DH // P           # 2 partition chunks over this core's head dims
    NB = N // 512          # 4 query blocks
    KB = N // P            # 16 key-position chunks

    with ExitStack() as ctx:
        tc = ctx.enter_context(tile.TileContext(nc))
        const = ctx.enter_context(tc.tile_pool(name="const", bufs=1))
        xpool = ctx.enter_context(tc.tile_pool(name="xpool", bufs=12))
        wpool = ctx.enter_context(tc.tile_pool(name="wpool", bufs=3))
        wopool = ctx.enter_context(tc.tile_pool(name="wopool", bufs=1))
        qkpool = ctx.enter_context(tc.tile_pool(name="qkpool", bufs=MC))
        vpool = ctx.enter_context(tc.tile_pool(name="vpool", bufs=KB))
        ptpool = ctx.enter_context(tc.tile_pool(name="ptpool", bufs=34))
        apool = ctx.enter_context(tc.tile_pool(name="apool", bufs=16))
        atpool = ctx.enter_context(tc.tile_pool(name="atpool", bufs=4))
        ypool = ctx.enter_context(tc.tile_pool(name="ypool", bufs=2))
        spool = ctx.enter_context(tc.tile_pool(name="spool", bufs=8))
        pp = ctx.enter_context(tc.tile_pool(name="pp", bufs=2, space="PSUM"))
        stq = ctx.enter_context(tc.tile_pool(name="stq", bufs=2, space="PSUM"))
        pss = ctx.enter_context(tc.tile_pool(name="pss", bufs=2, space="PSUM"))

        def load_wx(w_dram, x_dram, tag):
            """One packed DMA for the weight (4KB lines); x arrives
            position-major (all chunks' first halves, then second halves) so
            nb-outer projections start after only half the tensor has
            landed.  First chunk rides the sync hw queue, the rest gpsimd."""
            wt = wpool.tile([P, KC * DH], BF16, name=tag)
            nc.sync.dma_start(out=wt, in_=w_dram[:, :])
            ws = [wt[:, kc * DH:(kc + 1) * DH] for kc in range(KC)]
            xs = [xpool.tile([P, N], BF16, name="xs") for _ in range(KC)]
            for hh in range(4):
                for kc in range(KC):
                    eng = nc.sync if kc == 0 else nc.gpsimd
                    eng.dma_start(
                        out=xs[kc][:, hh * 512:(hh + 1) * 512],
                        in_=x_dram[kc * P:(kc + 1) * P, hh * 512:(hh + 1) * 512])
            return ws, xs

        # --- transposed projections: out[c][dd, n], eviction on DVE,
        # emitted one (nb, c) block at a time so the caller controls
        # interleaving with downstream work ---
        def proj_block(xs, w_c, o, nb, c, scale_sb, bias_sb):
            pt = pp.tile([P, 512], F32, name="pp")
            for kc in range(KC):
                nc.tensor.matmul(
                    pt,
                    lhsT=w_c[kc][:, c * P:(c + 1) * P],
                    rhs=xs[kc][:, nb * 512:(nb + 1) * 512],
                    start=(kc == 0), stop=(kc == KC - 1))
            ob = o[:, nb * 512:(nb + 1) * 512]
            if scale_sb is not None:
                nc.vector.tensor_scalar(
                    out=ob, in0=pt,
                    scalar1=scale_sb[:, c:c + 1],
                    scalar2=bias_sb[:, c:c + 1],
                    op0=MUL, op1=ADD)
            else:
                nc.vector.tensor_scalar_add(
                    out=ob, in0=pt, scalar1=bias_sb[:, c:c + 1])

        wk_c, xk = load_wx(wkP, xkT, "wk")
        sb_sb = const.tile([P, 3 * MC], F32, name="sbt")
        nc.sync.dma_start(out=sb_sb, in_=sbt[:, :])
        qs_sb = sb_sb[:, 0:MC]
        qb_sb = sb_sb[:, MC:2 * MC]
        kb_sb = sb_sb[:, 2 * MC:3 * MC]
        ident = const.tile([P, P], BF16)
        make_identity(nc, ident)

        wq_c, xq = load_wx(wqP, xqT, "wq")
        kT = [qkpool.tile([P, N], BF16, name="kt") for _ in range(MC)]
        for nb in range(NB):
            for c in range(MC):
                proj_block(xk, wk_c, kT[c], nb, c, None, kb_sb)
        qT = [qkpool.tile([P, N], BF16, name="qt") for _ in range(MC)]
        for nb in range(2):
            for c in range(MC):
                proj_block(xq, wq_c, qT[c], nb, c, qs_sb, qb_sb)
        wot = wopool.tile([P, MC * E], BF16, name="wo")
        nc.sync.dma_start(out=wot, in_=woP[:, :])
        wo_sb = [wot[:, c * E:(c + 1) * E] for c in range(MC)]

        wv_c, xv = load_wx(wvP, xvT, "wv")
        vb_ap = vbias[:]
        vb_bc = const.tile([P, DH], F32, name="vb")
        nc.gpsimd.dma_start(out=vb_bc, in_=bass.AP(
            tensor=vb_ap.tensor, offset=vb_ap.offset, ap=[[0, P]] + vb_ap.ap))

        # --- v in natural [pos, dim] layout, bf16, ones column at dim 64 ---
        v_sb = []

        def emit_v_chunk(m):
            vt = vpool.tile([P, HG, D + 1], BF16, name="vt")
            nc.gpsimd.memset(vt[:, :, D:D + 1], 1.0)
            pv = pp.tile([P, 512], F32, name="pp")[:, :DH]
            for kc in range(KC):
                nc.tensor.matmul(
                    pv,
                    lhsT=xv[kc][:, m * P:(m + 1) * P],
                    rhs=wv_c[kc],
                    start=(kc == 0), stop=(kc == KC - 1))
            nc.vector.tensor_add(
                out=vt[:, :, 0:D],
                in0=pv.rearrange("p (h d) -> p h d", h=HG),
                in1=vb_bc.rearrange("p (h d) -> p h d", h=HG))
            v_sb.append(vt)

        # --- attention + output projection ---
        units = [(qb, h) for qb in range(N // 1024) for h in range(HG)]
        a_tiles_all = {}
        pts_cur = {}

        def get_a_tiles(qb):
            if qb not in a_tiles_all:
                a_tiles_all[qb] = [apool.tile([P, DH], BF16, name="acc")
                                   for _ in range(8)]
            return a_tiles_all[qb]

        def emit_se_chunk(u, kc):
            qb, h = u
            c, off = divmod(h * D, P)
            get_a_tiles(qb)
            stp = stq.tile([P, 1024], F32, name="stq")
            for qh in range(2):
                q0 = qb * 1024 + qh * 512
                nc.tensor.matmul(
                    stp[:, qh * 512:(qh + 1) * 512],
                    lhsT=kT[c][off:off + D, kc * P:(kc + 1) * P],
                    rhs=qT[c][off:off + D, q0:q0 + 512],
                    start=True, stop=True)
            ptile = ptpool.tile([P, 1024], BF16, name="pt")
            nc.scalar.activation(out=ptile, in_=stp, func=AF.Exp)
            pts_cur.setdefault(u, []).append(ptile)

        def emit_av_group(u, ql):
            qb, h = u
            pts = pts_cur[u]
            av = pss.tile([P, D + 1], F32, name="pss")
            for kc in range(KB):
                nc.tensor.matmul(
                    av, lhsT=pts[kc][:, ql * P:(ql + 1) * P],
                    rhs=v_sb[kc][:, h, :],
                    start=(kc == 0), stop=(kc == KB - 1))
            rt = spool.tile([P, 1], F32, name="rt")
            nc.vector.reciprocal(out=rt, in_=av[:, D:D + 1])
            nc.vector.tensor_scalar_mul(
                out=get_a_tiles(qb)[ql][:, h * D:(h + 1) * D],
                in0=av[:, 0:D], scalar1=rt)
            if ql == 7:
                del pts_cur[u]

        def emit_oproj_group(qb, ql):
            a_tiles = a_tiles_all[qb]
            at = []
            for c2 in range(MC):
                tp = pss.tile([P, P], BF16, name="pss")
                nc.tensor.transpose(
                    tp, a_tiles[ql][:, c2 * P:(c2 + 1) * P], ident)
                att = atpool.tile([P, P], BF16, name="att")
                nc.scalar.copy(out=att, in_=tp)
                at.append(att)
            yt = ypool.tile([P, E], BF16, name="yt")
            for nn in range(2):
                py = pp.tile([P, 512], F32, name="pp")
                for c2 in range(MC):
                    nc.tensor.matmul(
                        py, lhsT=at[c2],
                        rhs=wo_sb[c2][:, nn * 512:(nn + 1) * 512],
                        start=(c2 == 0), stop=(c2 == MC - 1))
                nc.scalar.copy(out=yt[:, nn * 512:(nn + 1) * 512], in_=py)
            q0 = qb * 1024 + ql * P
            nc.sync.dma_start(out=y[q0:q0 + P, :], in_=yt)

        from collections import deque
        fill = deque()
        # Unit 0's S/exp chunks interleave with the Q-projection tail and
        # the V projection: exp gets fed as early as possible while the PE
        # runs the remaining projections between S chunks.
        tail = deque()
        for nb in range(2, NB):
            for c in range(MC):
                tail.append(lambda n=nb, cc=c: proj_block(
                    xq, wq_c, qT[cc], n, cc, qs_sb, qb_sb))
        for m in range(KB):
            tail.append(lambda mm=m: emit_v_chunk(mm))
        for kc in range(KB):
            emit_se_chunk(units[0], kc)
            if tail:
                tail.popleft()()
        while tail:
            tail.popleft()()
        for i in range(1, len(units) + 1):
            prev = units[i - 1]
            if prev[1] == HG - 1:
                for ql in range(8):
                    fill.append(lambda u=prev, q=ql: emit_av_group(u, q))
                    fill.append(lambda b=prev[0], q=ql: emit_oproj_group(b, q))
            else:
                for ql in range(8):
                    fill.append(lambda u=prev, q=ql: emit_av_group(u, q))
            if i < len(units):
                for j in range(KB):
                    emit_se_chunk(units[i], j)
                    if fill and (j % 2 == 1 or len(fill) > 10):
                        fill.popleft()()
            else:
                while fill:
                    fill.popleft()()

    _split_drain_waits(nc)
    return nc


_CACHE = {}


def _get_nc():
    if "nc" not in _CACHE:
        _CACHE["nc"] = _build()
    return _CACHE["nc"]


def kernel(query, key, value, Wq, bq, Wk, bk, Wv, bv, Wo, bo, gate):
    global LAST_RESULTS
    query = np.asarray(query, np.float32)
    key = np.asarray(key, np.float32)
    value = np.asarray(value, np.float32)
    Wq = np.asarray(Wq, np.float32)
    Wk = np.asarray(Wk, np.float32)
    Wv = np.asarray(Wv, np.float32)
    Wo = np.asarray(Wo, np.float32)
    bq = np.asarray(bq, np.float32)
    bk = np.asarray(bk, np.float32)
    bv = np.asarray(bv, np.float32)
    bo = np.asarray(bo, np.float32)
    gate = np.asarray(gate, np.float32)

    scale_h = (1.0 / (1.0 + np.exp(-gate.astype(np.float64)))
               / math.sqrt(D)).astype(np.float32)

    xq_b = [np.ascontiguousarray(query[b].T) for b in range(B)]
    xk_b = [np.ascontiguousarray(key[b].T) for b in range(B)]
    xv_b = [np.ascontiguousarray(value[b].T) for b in range(B)]

    KC = E // P
    MC = DH // P

    def pack_w(wT, cols):  # [E, DH]-like -> [P, cols/P * DH-ish] chunk-major
        kc = wT.shape[0] // P
        return np.ascontiguousarray(
            wT.reshape(kc, P, wT.shape[1]).transpose(1, 0, 2).reshape(P, -1))

    in_maps = []
    for core in range(NCORES):
        b, g = divmod(core, GROUPS)
        rows = slice(g * DH, (g + 1) * DH)
        qs = np.repeat(scale_h[g * HG:(g + 1) * HG], D)
        sbt = np.concatenate([
            qs.reshape(MC, P).T,
            (bq[rows] * qs).reshape(MC, P).T,
            bk[rows].reshape(MC, P).T,
        ], axis=1)
        in_maps.append({
            "xqT": xq_b[b], "xkT": xk_b[b], "xvT": xv_b[b],
            "wqP": pack_w(Wq[rows].T, KC),
            "wkP": pack_w(Wk[rows].T, KC),
            "wvP": pack_w(Wv[rows].T, KC),
            "woP": pack_w(Wo[:, rows].T, MC),
            "sbt": np.ascontiguousarray(sbt, np.float32),
            "vbias": np.ascontiguousarray(bv[rows]),
        })

    from concourse import mybir as _mb
    bf = _mb.dt.np(_mb.dt.bfloat16)
    for m in in_maps:
        for k in ("xqT", "xkT", "xvT", "wqP", "wkP", "wvP", "woP"):
            m[k] = m[k].astype(bf)
    res = run_bass_kernel_spmd(_get_nc(), in_maps, list(range(NCORES)),
                               trace=TRACE)
    LAST_RESULTS = res
    out = np.empty((B, N, E), np.float32)
    for b in range(B):
        acc = res.results[b * GROUPS]["y"].astype(np.float32).copy()
        for g in range(1, GROUPS):
            acc += res.results[b * GROUPS + g]["y"].astype(np.float32)
        out[b] = acc + bo
    return out


# revision 33
# speedup vs baseline: 1.0089x; 1.0089x over previous
"""Gated multi-head attention on 8 NeuronCores.

Sharding (hardcoded): core c -> (batch b = c // 4, head-group g = c % 4).
Data-parallel over B=2, tensor-parallel over the 16 heads in groups of 4.
Each core computes its 4 heads' attention plus the corresponding slice of
the output projection; the host sums the 4 head-group partials per batch
(bf16 on the wire, f32 accumulate) and adds the output bias.

Per-core kernel (all matmuls bf16):
  qT[256,2048] = (Wq_slice.T).T @ x_q.T   (gate sigmoid/sqrt(D) + bias folded
                                           into the PSUM->SBUF eviction, which
                                           runs on the vector engine so the
                                           scalar engine does exp only)
  kT[256,2048], v[2048,256] likewise (v in natural [pos, dim] layout, bf16,
                                      with a ones-column appended per head)
  per head h, per 512-wide query block:
    S^T[k,q] = kT_h.T-chunks @ qT_h      (K=64 contraction)
    P^T = exp(S^T)                       (no max-subtraction: logits ~ +-4)
    acc[q,65] = P^T-chunks.T @ [V_h | 1] (bf16; col 64 = softmax denominator)
    A[q, h*64:...] = acc[:, :64] * recip(acc[:, 64])
  y[q,1024] = A @ Wo_slice.T  via PE-transpose of A then bf16 matmul

Schedule: V-projection chunks are interleaved with unit-0's S^T chunks so
exp starts ~15us earlier; AV groups of earlier units fill PE gaps during
later units' exp phases; O-proj groups interleave with the last head's AV
groups so y stores drain early.
"""

import math
from contextlib import ExitStack

import numpy as np

import concourse.bass as bass
import concourse.tile as tile
from concourse import mybir
from concourse.bass_utils import run_bass_kernel_spmd
from concourse.masks import make_identity

B = 2
N = 2048
E = 1024
H = 16
D = 64
NCORES = 8
GROUPS = NCORES // B      # head-groups per batch
HG = H // GROUPS          # heads per core
DH = HG * D               # 256 head-dims per core
P = 128

F32 = mybir.dt.float32
BF16 = mybir.dt.bfloat16
AF = mybir.ActivationFunctionType
MUL = mybir.AluOpType.mult
ADD = mybir.AluOpType.add

TRACE = False
LAST_RESULTS = None


def _split_drain_waits(nc):
    """The installed walrus build accepts only ONE sync-wait per instruction
    (one NEURON_ISA_TPB_EVENTS slot), but Tile emits several on drains,
    matmuls, etc.  Hoist all but the last wait onto dedicated single-wait
    NOPs ahead of the instruction on the same engine (the lowering newer
    walrus performs itself)."""
    n = 0
    for fn in nc.m.functions:
        for bb in fn.blocks:
            insts = bb.instructions
            idx = 0
            while idx < len(insts):
                inst = insts[idx]
                si = inst.sync_info
                if si is not None and len(si.on_wait) > 1:
                    waits = list(si.on_wait)
                    nops = []
                    for w in waits[:-1]:
                        n += 1
                        nop = mybir.InstNoOp(
                            name=f"waitsplit-{n}",
                            engine=inst.engine,
                            sync_info=mybir.SyncInfo(on_wait=[w], on_update=[]),
                            bass_nofuse=True,
                        )
                        nc.register_instruction(nop)
                        nops.append(nop)
                    inst.sync_info = mybir.SyncInfo(
                        on_wait=[waits[-1]], on_update=list(si.on_update))
                    insts[idx:idx] = nops
                    idx += len(nops)
                idx += 1
    return n


def _build():
    nc = bass.Bass()
    # Weights are pre-packed on the host, chunk-major: [128, KC*DH] where
    # column kc*DH+j holds W.T[kc*128+p, j] -- each SBUF partition row is
    # one contiguous 4KB DMA line instead of KC strided 512B lines.
    xqT = nc.dram_tensor("xqT", [E, N], BF16, kind="ExternalInput")
    xkT = nc.dram_tensor("xkT", [E, N], BF16, kind="ExternalInput")
    xvT = nc.dram_tensor("xvT", [E, N], BF16, kind="ExternalInput")
    wqP = nc.dram_tensor("wqP", [P, (E // P) * DH], BF16, kind="ExternalInput")
    wkP = nc.dram_tensor("wkP", [P, (E // P) * DH], BF16, kind="ExternalInput")
    wvP = nc.dram_tensor("wvP", [P, (E // P) * DH], BF16, kind="ExternalInput")
    woP = nc.dram_tensor("woP", [P, (DH // P) * E], BF16, kind="ExternalInput")
    # scale/bias table, [128, 3*MC]: cols [qscale | qbias | kbias] x MC
    sbt = nc.dram_tensor("sbt", [P, 3 * (DH // P)], F32, kind="ExternalInput")
    vbias = nc.dram_tensor("vbias", [DH], F32, kind="ExternalInput")
    y = nc.dram_tensor("y", [N, E], BF16, kind="ExternalOutput")

    KC = E // P            # 8 contraction chunks over the embed dim
    MC = DH // P           # 2 partition chunks over this core's head dims
    NB = N // 512          # 4 query blocks
    KB = N // P            # 16 key-position chunks

    with ExitStack() as ctx:
        tc = ctx.enter_context(tile.TileContext(nc))
        const = ctx.enter_context(tc.tile_pool(name="const", bufs=1))
        xpool = ctx.enter_context(tc.tile_pool(name="xpool", bufs=12))
        wpool = ctx.enter_context(tc.tile_pool(name="wpool", bufs=3))
        wopool = ctx.enter_context(tc.tile_pool(name="wopool", bufs=1))
        qkpool = ctx.enter_context(tc.tile_pool(name="qkpool", bufs=MC))
        vpool = ctx.enter_context(tc.tile_pool(name="vpool", bufs=KB))
        ptpool = ctx.enter_context(tc.tile_pool(name="ptpool", bufs=34))
        apool = ctx.enter_context(tc.tile_pool(name="apool", bufs=16))
        atpool = ctx.enter_context(tc.tile_pool(name="atpool", bufs=4))
        ypool = ctx.enter_context(tc.tile_pool(name="ypool", bufs=2))
        spool = ctx.enter_context(tc.tile_pool(name="spool", bufs=8))
        pp = ctx.enter_context(tc.tile_pool(name="pp", bufs=2, space="PSUM"))
        stq = ctx.enter_context(tc.tile_pool(name="stq", bufs=2, space="PSUM"))
        pss = ctx.enter_context(tc.tile_pool(name="pss", bufs=2, space="PSUM"))

        def load_wx(w_dram, x_dram, tag):
            """One packed DMA for the weight (4KB lines); x arrives
            position-major (all chunks' first halves, then second halves) so
            nb-outer projections start after only half the tensor has
            landed.  First chunk rides the sync hw queue, the rest gpsimd."""
            wt = wpool.tile([P, KC * DH], BF16, name=tag)
            nc.sync.dma_start(out=wt, in_=w_dram[:, :])
            ws = [wt[:, kc * DH:(kc + 1) * DH] for kc in range(KC)]
            xs = [xpool.tile([P, N], BF16, name="xs") for _ in range(KC)]
            for hh in range(4):
                for kc in range(KC):
                    eng = nc.sync if kc == 0 else nc.gpsimd
                    eng.dma_start(
                        out=xs[kc][:, hh * 512:(hh + 1) * 512],
                        in_=x_dram[kc * P:(kc + 1) * P, hh * 512:(hh + 1) * 512])
            return ws, xs

        # --- transposed projections: out[c][dd, n], eviction on DVE,
        # emitted one (nb, c) block at a time so the caller controls
        # interleaving with downstream work ---
        def proj_block(xs, w_c, o, nb, c, scale_sb, bias_sb):
            pt = pp.tile([P, 512], F32, name="pp")
            for kc in range(KC):
                nc.tensor.matmul(
                    pt,
                    lhsT=w_c[kc][:, c * P:(c + 1) * P],
                    rhs=xs[kc][:, nb * 512:(nb + 1) * 512],
                    start=(kc == 0), stop=(kc == KC - 1))
            ob = o[:, nb * 512:(nb + 1) * 512]
            if scale_sb is not None:
                nc.vector.tensor_scalar(
                    out=ob, in0=pt,
                    scalar1=scale_sb[:, c:c + 1],
                    scalar2=bias_sb[:, c:c + 1],
                    op0=MUL, op1=ADD)
            else:
                nc.vector.tensor_scalar_add(
                    out=ob, in0=pt, scalar1=bias_sb[:, c:c + 1])

        wk_c, xk = load_wx(wkP, xkT, "wk")
        sb_sb = const.tile([P, 3 * MC], F32, name="sbt")
        nc.sync.dma_start(out=sb_sb, in_=sbt[:, :])
        qs_sb = sb_sb[:, 0:MC]
        qb_sb = sb_sb[:, MC:2 * MC]
        kb_sb = sb_sb[:, 2 * MC:3 * MC]
        ident = const.tile([P, P], F32)
        make_identity(nc, ident)

        wq_c, xq = load_wx(wqP, xqT, "wq")
        kT = [qkpool.tile([P, N], BF16, name="kt") for _ in range(MC)]
        for nb in range(NB):
            for c in range(MC):
                proj_block(xk, wk_c, kT[c], nb, c, None, kb_sb)
        qT = [qkpool.tile([P, N], BF16, name="qt") for _ in range(MC)]
        for nb in range(2):
            for c in range(MC):
                proj_block(xq, wq_c, qT[c], nb, c, qs_sb, qb_sb)
        wot = wopool.tile([P, MC * E], BF16, name="wo")
        nc.sync.dma_start(out=wot, in_=woP[:, :])
        wo_sb = [wot[:, c * E:(c + 1) * E] for c in range(MC)]

        wv_c, xv = load_wx(wvP, xvT, "wv")
        vb_ap = vbias[:]
        vb_bc = const.tile([P, DH], F32, name="vb")
        nc.gpsimd.dma_start(out=vb_bc, in_=bass.AP(
            tensor=vb_ap.tensor, offset=vb_ap.offset, ap=[[0, P]] + vb_ap.ap))

        # --- v in natural [pos, dim] layout, bf16, ones column at dim 64 ---
        v_sb = []

        def emit_v_chunk(m):
            vt = vpool.tile([P, HG, D + 1], BF16, name="vt")
            nc.gpsimd.memset(vt[:, :, D:D + 1], 1.0)
            pv = pp.tile([P, 512], F32, name="pp")[:, :DH]
            for kc in range(KC):
                nc.tensor.matmul(
                    pv,
                    lhsT=xv[kc][:, m * P:(m + 1) * P],
                    rhs=wv_c[kc],
                    start=(kc == 0), stop=(kc == KC - 1))
            nc.vector.tensor_add(
                out=vt[:, :, 0:D],
                in0=pv.rearrange("p (h d) -> p h d", h=HG),
                in1=vb_bc.rearrange("p (h d) -> p h d", h=HG))
            v_sb.append(vt)

        # --- attention + output projection ---
        units = [(qb, h) for qb in range(N // 1024) for h in range(HG)]
        a_tiles_all = {}
        pts_cur = {}

        def get_a_tiles(qb):
            if qb not in a_tiles_all:
                a_tiles_all[qb] = [apool.tile([P, DH], F32, name="acc")
                                   for _ in range(8)]
            return a_tiles_all[qb]

        def emit_se_chunk(u, kc):
            qb, h = u
            c, off = divmod(h * D, P)
            get_a_tiles(qb)
            stp = stq.tile([P, 1024], F32, name="stq")
            for qh in range(2):
                q0 = qb * 1024 + qh * 512
                nc.tensor.matmul(
                    stp[:, qh * 512:(qh + 1) * 512],
                    lhsT=kT[c][off:off + D, kc * P:(kc + 1) * P],
                    rhs=qT[c][off:off + D, q0:q0 + 512],
                    start=True, stop=True)
            ptile = ptpool.tile([P, 1024], BF16, name="pt")
            nc.scalar.activation(out=ptile, in_=stp, func=AF.Exp)
            pts_cur.setdefault(u, []).append(ptile)

        def emit_av_group(u, ql):
            qb, h = u
            pts = pts_cur[u]
            av = pss.tile([P, D + 1], F32, name="pss")
            for kc in range(KB):
                nc.tensor.matmul(
                    av, lhsT=pts[kc][:, ql * P:(ql + 1) * P],
                    rhs=v_sb[kc][:, h, :],
                    start=(kc == 0), stop=(kc == KB - 1))
            rt = spool.tile([P, 1], F32, name="rt")
            nc.vector.reciprocal(out=rt, in_=av[:, D:D + 1])
            nc.vector.tensor_scalar_mul(
                out=get_a_tiles(qb)[ql][:, h * D:(h + 1) * D],
                in0=av[:, 0:D], scalar1=rt)
            if ql == 7:
                del pts_cur[u]

        def emit_oproj_group(qb, ql):
            a_tiles = a_tiles_all[qb]
            at = []
            for c2 in range(MC):
                tp = pss.tile([P, P], F32, name="pss")
                nc.tensor.transpose(
                    tp, a_tiles[ql][:, c2 * P:(c2 + 1) * P], ident)
                att = atpool.tile([P, P], BF16, name="att")
                nc.scalar.copy(out=att, in_=tp)
                at.append(att)
            yt = ypool.tile([P, E], BF16, name="yt")
            for nn in range(2):
                py = pp.tile([P, 512], F32, name="pp")
                for c2 in range(MC):
                    nc.tensor.matmul(
                        py, lhsT=at[c2],
                        rhs=wo_sb[c2][:, nn * 512:(nn + 1) * 512],
                        start=(c2 == 0), stop=(c2 == MC - 1))
                nc.scalar.copy(out=yt[:, nn * 512:(nn + 1) * 512], in_=py)
            q0 = qb * 1024 + ql * P
            nc.sync.dma_start(out=y[q0:q0 + P, :], in_=yt)

        from collections import deque
        fill = deque()
        # Unit 0's S/exp chunks interleave with the Q-projection tail and
        # the V projection: exp gets fed as early as possible while the PE
        # runs the remaining projections between S chunks.
        tail = deque()
        for nb in range(2, NB):
            for c in range(MC):
                tail.append(lambda n=nb, cc=c: proj_block(
                    xq, wq_c, qT[cc], n, cc, qs_sb, qb_sb))
        for m in range(KB):
            tail.append(lambda mm=m: emit_v_chunk(mm))
        for kc in range(KB):
            emit_se_chunk(units[0], kc)
            if tail:
                tail.popleft()()
        while tail:
            tail.popleft()()
        for i in range(1, len(units) + 1):
            prev = units[i - 1]
            if prev[1] == HG - 1:
                for ql in range(8):
                    fill.append(lambda u=prev, q=ql: emit_av_group(u, q))
                    fill.append(lambda b=prev[0], q=ql: emit_oproj_group(b, q))
            else:
                for ql in range(8):
                    fill.append(lambda u=prev, q=ql: emit_av_group(u, q))
            if i < len(units):
                for j in range(KB):
                    emit_se_chunk(units[i], j)
                    if fill and (j % 2 == 1 or len(fill) > 10):
                        fill.popleft()()
            else:
                while fill:
                    fill.popleft()()

    _split_drain_waits(nc)
    return nc


_CACHE = {}


def _get_nc():
    if "nc" not in _CACHE:
        _CACHE["nc"] = _build()
    return _CACHE["nc"]


def kernel(query, key, value, Wq, bq, Wk, bk, Wv, bv, Wo, bo, gate):
    global LAST_RESULTS
    query = np.asarray(query, np.float32)
    key = np.asarray(key, np.float32)
    value = np.asarray(value, np.float32)
    Wq = np.asarray(Wq, np.float32)
    Wk = np.asarray(Wk, np.float32)
    Wv = np.asarray(Wv, np.float32)
    Wo = np.asarray(Wo, np.float32)
    bq = np.asarray(bq, np.float32)
    bk = np.asarray(bk, np.float32)
    bv = np.asarray(bv, np.float32)
    bo = np.asarray(bo, np.float32)
    gate = np.asarray(gate, np.float32)

    scale_h = (1.0 / (1.0 + np.exp(-gate.astype(np.float64)))
               / math.sqrt(D)).astype(np.float32)

    xq_b = [np.ascontiguousarray(query[b].T) for b in range(B)]
    xk_b = [np.ascontiguousarray(key[b].T) for b in range(B)]
    xv_b = [np.ascontiguousarray(value[b].T) for b in range(B)]

    KC = E // P
    MC = DH // P

    def pack_w(wT, cols):  # [E, DH]-like -> [P, cols/P * DH-ish] chunk-major
        kc = wT.shape[0] // P
        return np.ascontiguousarray(
            wT.reshape(kc, P, wT.shape[1]).transpose(1, 0, 2).reshape(P, -1))

    in_maps = []
    for core in range(NCORES):
        b, g = divmod(core, GROUPS)
        rows = slice(g * DH, (g + 1) * DH)
        qs = np.repeat(scale_h[g * HG:(g + 1) * HG], D)
        sbt = np.concatenate([
            qs.reshape(MC, P).T,
            (bq[rows] * qs).reshape(MC, P).T,
            bk[rows].reshape(MC, P).T,
        ], axis=1)
        in_maps.append({
            "xqT": xq_b[b], "xkT": xk_b[b], "xvT": xv_b[b],
            "wqP": pack_w(Wq[rows].T, KC),
            "wkP": pack_w(Wk[rows].T, KC),
            "wvP": pack_w(Wv[rows].T, KC),
            "woP": pack_w(Wo[:, rows].T, MC),
            "sbt": np.ascontiguousarray(sbt, np.float32),
            "vbias": np.ascontiguousarray(bv[rows]),
        })

    from concourse import mybir as _mb
    bf = _mb.dt.np(_mb.dt.bfloat16)
    for m in in_maps:
        for k in ("xqT", "xkT", "xvT", "wqP", "wkP", "wvP", "woP"):
            m[k] = m[k].astype(bf)
    res = run_bass_kernel_spmd(_get_nc(), in_maps, list(range(NCORES)),
                               trace=TRACE)
    LAST_RESULTS = res
    out = np.empty((B, N, E), np.float32)
    for b in range(B):
        acc = res.results[b * GROUPS]["y"].astype(np.float32).copy()
        for g in range(1, GROUPS):
            acc += res.results[b * GROUPS + g]["y"].astype(np.float32)
        out[b] = acc + bo
    return out
